# revision 3
# baseline (speedup 1.0000x reference)
"""DGCNN (3x DynamicEdgeConv, kNN=20) Trainium2 Bass kernel.

Self-contained: `kernel(**inputs) -> np.ndarray` takes the full inputs from
setup_inputs() (pos [8,4096,3] + 9 weight/bias pairs) and returns [8,4096,64].

Sharding: data-parallel over batch B=8 -> one point cloud per NeuronCore,
weights replicated. Each core runs the identical program on its slice.

Per-core, per-layer pipeline (N=4096 points, feature dim D in {3,64}, H=64):
  phase 0: X2T = 2*X^T in SBUF [64,4096]; negsq2 = -2*||x_j||^2 row;
           U = x@(W1a-W1b)+b1 (point-major, SBUF), V = x@W1b -> DRAM [4096,64]
  per row-tile t (128 points):
    S = 4 x_i.x_j - 2 sq_j  (PE, accumulating -2sq row; row-monotone == -dist)
    top-20: per-128-chunk top-8 via DVE max/max_index -> 256 candidates,
            3x (max8 + match_replace) -> 24 winners, positions via max_index,
            per-row candidate-index lookup via gpsimd indirect_copy + 16
            strided-partition diagonal DMAs -> idx [128,24] (first 20 valid)
    h1 = leaky(U_i + V_j): prefill U (ACT), 20x indirect DMA gather with
            CCE-add, Prelu(alpha=0.2)
    h1 -> 20 PE transposes -> h1T [64, 20*128] (edge k-major)
    h2T = Prelu(W2^T@h1T + b2), h3T = Prelu(W3^T@h2T + b3)   (PE + ACT)
    out tile = max over k (DVE strided reduce) -> next layer's X2T
"""
import numpy as np

import concourse.bass as bass
import concourse.bacc as bacc
import concourse.mybir as mybir
import concourse.tile as tile
from concourse.bass_utils import run_bass_kernel_spmd
from concourse.masks import make_identity

F32 = mybir.dt.float32
U16 = mybir.dt.uint16
U32 = mybir.dt.uint32
AF = mybir.ActivationFunctionType
ALU = mybir.AluOpType

B = 8
N = 4096
P = 128
NT = N // P            # 32 row tiles
CS = 128               # top-k chunk size
NCH = N // CS          # 32 chunks
K = 20
H = 64
SLOPE = 0.2
NEG = -3.0e38
NSW = 4                # SWDGE queues for the edge gathers

_CACHE = {}


def _gather_q(nc, q, **kw):
    bi = nc.gpsimd.indirect_dma_start(**kw)
    if q:
        bi.ins.queue = f"qPoolDynamic{q}"
    return bi


def _build_layer(nc, tc, g, li, d_in, x2t, x2t_next, w1_d, b1_d, w2_d, b2_d,
                 w3_d, b3_d, v_d, out_d=None):
    """Emit one EdgeConv layer. x2t holds 2*X^T (rows 0..d_in-1 valid).
    Writes 2*X_next^T into x2t_next, or the final output to out_d (layer 3)."""
    sb = g["sb"]
    pp_s, pp_tp, pp_h = g["pp_s"], g["pp_tp"], g["pp_h"]
    ident, ones1, alpha64, alpha128, base = (
        g["ident"], g["ones1"], g["alpha64"], g["alpha128"], g["base"])

    # ---- phase 0: weights ----
    wa = sb.tile([d_in, H], F32, tag="wa")
    wb = sb.tile([d_in, H], F32, tag="wb")
    nc.sync.dma_start(wa[:], w1_d[0:d_in, :])
    nc.sync.dma_start(wb[:], w1_d[d_in:2 * d_in, :])
    wd2 = sb.tile([d_in, H], F32, tag="wd2")
    nc.vector.tensor_tensor(out=wd2[:], in0=wa[:], in1=wb[:], op=ALU.subtract)
    nc.vector.tensor_scalar_mul(wd2[:], wd2[:], 0.5)
    wb2 = sb.tile([d_in, H], F32, tag="wb2")
    nc.vector.tensor_scalar_mul(wb2[:], wb[:], 0.5)
    w2 = sb.tile([H, H], F32, tag="w2")
    nc.sync.dma_start(w2[:], w2_d[:])
    w3 = sb.tile([H, H], F32, tag="w3")
    nc.sync.dma_start(w3[:], w3_d[:])
    b1r = sb.tile([1, H], F32, tag="b1r")
    nc.sync.dma_start(b1r[:], b1_d[:].unsqueeze(0))
    b2c = sb.tile([H, 1], F32, tag="b2c")
    nc.sync.dma_start(b2c[:], b2_d[:].unsqueeze(1))
    b3c = sb.tile([H, 1], F32, tag="b3c")
    nc.sync.dma_start(b3c[:], b3_d[:].unsqueeze(1))

    # ---- phase 0: negsq2 = -2*sq (from X2T: colsum(X2T^2) = 4 sq) ----
    xsq = g["s_pool"].tile([P, N], F32, tag="s")   # borrow an S buffer
    nc.scalar.activation(out=xsq[0:d_in, :], in_=x2t[0:d_in, :], func=AF.Square)
    onescol = sb.tile([d_in, 1], F32, tag="ones")
    nc.vector.memset(onescol[:], 1.0)
    negsq2 = sb.tile([1, N], F32, tag="negsq")
    for c in range(N // 512):
        ps = pp_tp.tile([1, 512], F32, tag="tp")
        nc.tensor.matmul(out=ps[:], lhsT=onescol[:], rhs=xsq[0:d_in, c * 512:(c + 1) * 512],
                         start=True, stop=True)
        nc.scalar.activation(out=negsq2[:, c * 512:(c + 1) * 512], in_=ps[:],
                             func=AF.Copy, scale=-0.5)

    # ---- phase 0: U (point-major SBUF) and V (point-major DRAM) ----
    u_sb = sb.tile([P, NT * H], F32, tag="u")
    vbuf = sb.tile([P, NT * H], F32, tag="vbuf")
    for t in range(NT):
        lhs = x2t[0:d_in, t * P:(t + 1) * P]
        pu = pp_tp.tile([P, H], F32, tag="tp")
        nc.tensor.matmul(out=pu[:], lhsT=lhs, rhs=wd2[:], start=True, stop=False)
        nc.tensor.matmul(out=pu[:], lhsT=ones1[:, 0:P], rhs=b1r[:], start=False, stop=True)
        nc.scalar.copy(out=u_sb[:, t * H:(t + 1) * H], in_=pu[:])
        pv = pp_tp.tile([P, H], F32, tag="tp")
        nc.tensor.matmul(out=pv[:], lhsT=lhs, rhs=wb2[:], start=True, stop=True)
        nc.scalar.copy(out=vbuf[:, t * H:(t + 1) * H], in_=pv[:])
    nc.sync.dma_start(v_d[:].rearrange("(t p) f -> p t f", p=P),
                      vbuf[:].rearrange("p (t f) -> p t f", f=H))

    # ---- per row-tile ----
    for t in range(NT):
        # distance tile S [128, N] (bigger = closer)
        s_sb = g["s_pool"].tile([P, N], F32, tag="s")
        for c in range(N // 512):
            ps = pp_s.tile([P, 512], F32, tag="dist")
            nc.tensor.matmul(out=ps[:], lhsT=x2t[0:d_in, t * P:(t + 1) * P],
                             rhs=x2t[0:d_in, c * 512:(c + 1) * 512],
                             start=True, stop=False)
            nc.tensor.matmul(out=ps[:], lhsT=ones1[:, 0:P],
                             rhs=negsq2[:, c * 512:(c + 1) * 512],
                             start=False, stop=True)
            nc.scalar.copy(out=s_sb[:, c * 512:(c + 1) * 512], in_=ps[:])

        # top-k stage 1: per-chunk top-8 values + indices
        cands = g["k_pool"].tile([P, NCH * 8], F32, tag="cands")
        glob = g["k_pool"].tile([P, NCH * 8], U32, tag="glob")
        for c in range(NCH):
            sl = s_sb[:, c * CS:(c + 1) * CS]
            nc.vector.max(out=cands[:, c * 8:(c + 1) * 8], in_=sl)
            nc.vector.max_index(out=glob[:, c * 8:(c + 1) * 8],
                                in_max=cands[:, c * 8:(c + 1) * 8], in_values=sl)
        nc.vector.tensor_tensor(out=glob[:], in0=glob[:], in1=base[:], op=ALU.add)

        # stage 2: global top-24 of candidates
        winners = g["k_pool"].tile([P, 24], F32, tag="win")
        pos = g["k_pool"].tile([P, 24], U16, tag="pos")
        work1 = g["k_pool"].tile([P, NCH * 8], F32, tag="wk1")
        work2 = g["k_pool"].tile([P, NCH * 8], F32, tag="wk2")
        nc.vector.max(out=winners[:, 0:8], in_=cands[:])
        nc.vector.match_replace(out=work1[:], in_to_replace=winners[:, 0:8],
                                in_values=cands[:], imm_value=NEG)
        nc.vector.max(out=winners[:, 8:16], in_=work1[:])
        nc.vector.match_replace(out=work2[:], in_to_replace=winners[:, 8:16],
                                in_values=work1[:], imm_value=NEG)
        nc.vector.max(out=winners[:, 16:24], in_=work2[:])
        for r in range(3):
            nc.vector.max_index(out=pos[:, r * 8:(r + 1) * 8],
                                in_max=winners[:, r * 8:(r + 1) * 8],
                                in_values=cands[:])

        # per-row index lookup: look[p, s*16+r] = glob[p, pos[row16q+r, s]]
        look = g["k_pool"].tile([P, 24 * 16], U32, tag="look")
        nc.gpsimd.indirect_copy(out=look[:], data=glob[:], idxs=pos[:],
                                i_know_ap_gather_is_preferred=True)
        idxt = g["k_pool"].tile([P, 24], U32, tag="idxt")
        for r in range(16):
            nc.sync.dma_start(idxt[r:P:16, 0:24], look[r:P:16, r:24 * 16:16])

        # edge features: h1 = leaky(U_i + V_j)
        h1 = g["h1_pool"].tile([P, K * H], F32, tag="h1")
        nc.scalar.copy(out=h1[:].rearrange("p (k f) -> p k f", k=K),
                       in_=u_sb[:, t * H:(t + 1) * H].unsqueeze(1).to_broadcast([P, K, H]))
        for k in range(K):
            _gather_q(nc, k % NSW,
                      out=h1[:, k * H:(k + 1) * H], out_offset=None,
                      in_=v_d[:],
                      in_offset=bass.IndirectOffsetOnAxis(ap=idxt[:, k:k + 1], axis=0),
                      compute_op=ALU.add)
        nc.scalar.activation(out=h1[:], in_=h1[:], func=AF.Prelu, alpha=alpha128[:])

        # transpose to edge-major h1T [64, k*128+i]
        h1t = g["ht_pool"].tile([H, K * P], F32, tag="ht")
        for kc in range(5):
            pt = pp_tp.tile([H, 512], F32, tag="tp")
            for j in range(4):
                k = kc * 4 + j
                nc.tensor.transpose(out=pt[:, j * P:(j + 1) * P],
                                    in_=h1[:, k * H:(k + 1) * H], identity=ident[:])
            nc.scalar.copy(out=h1t[:, kc * 512:(kc + 1) * 512], in_=pt[:])

        # MLP layers 2, 3 (feature-major, edges on the free axis)
        h2t = g["ht_pool"].tile([H, K * P], F32, tag="ht")
        for e in range(5):
            ph = pp_h.tile([H, 512], F32, tag="h")
            nc.tensor.matmul(out=ph[:], lhsT=w2[:], rhs=h1t[:, e * 512:(e + 1) * 512],
                             start=True, stop=True)
            nc.scalar.activation(out=h2t[:, e * 512:(e + 1) * 512], in_=ph[:],
                                 func=AF.Prelu, bias=b2c[:], alpha=alpha64[:])
        h3t = g["ht_pool"].tile([H, K * P], F32, tag="ht")
        for e in range(5):
            ph = pp_h.tile([H, 512], F32, tag="h")
            nc.tensor.matmul(out=ph[:], lhsT=w3[:], rhs=h2t[:, e * 512:(e + 1) * 512],
                             start=True, stop=True)
            nc.scalar.activation(out=h3t[:, e * 512:(e + 1) * 512], in_=ph[:],
                                 func=AF.Prelu, bias=b3c[:], alpha=alpha64[:])

        # aggregate: max over k (innermost stride-128 axis)
        ftile = g["f_pool"].tile([H, P], F32, tag="f")
        nc.vector.tensor_reduce(out=ftile[:],
                                in_=h3t[:].rearrange("h (k i) -> h i k", k=K),
                                axis=mybir.AxisListType.X, op=ALU.max)
        if out_d is None:
            nc.scalar.mul(out=x2t_next[0:H, t * P:(t + 1) * P], in_=ftile[:], mul=2.0)
        else:
            po = pp_tp.tile([P, H], F32, tag="tp")
            nc.tensor.transpose(out=po[:], in_=ftile[:], identity=ident[0:H, 0:H])
            nc.scalar.copy(out=g["vbuf_view"][:, t * H:(t + 1) * H], in_=po[:])
    if out_d is not None:
        nc.sync.dma_start(out_d[:].rearrange("(t p) f -> p t f", p=P),
                          g["vbuf_view"][:].rearrange("p (t f) -> p t f", f=H))


def build():
    nc = bacc.Bacc("TRN2", target_bir_lowering=False, debug=False,
                   num_swdge_queues=NSW)
    pos_d = nc.dram_tensor("pos", [N, 3], F32, kind="ExternalInput")
    wnames = {}
    for li, pfx in enumerate(("1", "2", "3")):
        d2 = 6 if li == 0 else 128
        wnames[f"w{pfx}1"] = nc.dram_tensor(f"w{pfx}1", [d2, H], F32, kind="ExternalInput")
        wnames[f"w{pfx}2"] = nc.dram_tensor(f"w{pfx}2", [H, H], F32, kind="ExternalInput")
        wnames[f"w{pfx}3"] = nc.dram_tensor(f"w{pfx}3", [H, H], F32, kind="ExternalInput")
        for j in ("1", "2", "3"):
            wnames[f"b{pfx}{j}"] = nc.dram_tensor(f"b{pfx}{j}", [H], F32, kind="ExternalInput")
    out_d = nc.dram_tensor("out", [N, H], F32, kind="ExternalOutput")
    v_ds = [nc.dram_tensor(f"vtab{li}", [N, H], F32) for li in range(3)]

    with tile.TileContext(nc) as tc:
        with tc.tile_pool(name="sb", bufs=1) as sb, \
             tc.tile_pool(name="s_pool", bufs=2) as s_pool, \
             tc.tile_pool(name="k_pool", bufs=2) as k_pool, \
             tc.tile_pool(name="h1_pool", bufs=2) as h1_pool, \
             tc.tile_pool(name="ht_pool", bufs=4) as ht_pool, \
             tc.tile_pool(name="f_pool", bufs=2) as f_pool, \
             tc.tile_pool(name="pp_s", bufs=3, space="PSUM") as pp_s, \
             tc.tile_pool(name="pp_tp", bufs=2, space="PSUM") as pp_tp, \
             tc.tile_pool(name="pp_h", bufs=2, space="PSUM") as pp_h:

            g = dict(sb=sb, s_pool=s_pool, k_pool=k_pool, h1_pool=h1_pool,
                     ht_pool=ht_pool, f_pool=f_pool,
                     pp_s=pp_s, pp_tp=pp_tp, pp_h=pp_h)

            ident = sb.tile([P, P], F32, tag="ident")
            make_identity(nc, ident)
            g["ident"] = ident
            ones1 = sb.tile([1, P], F32, tag="ones1")
            nc.vector.memset(ones1[:], 1.0)
            g["ones1"] = ones1
            alpha64 = sb.tile([H, 1], F32, tag="alpha64")
            nc.vector.memset(alpha64[:], SLOPE)
            g["alpha64"] = alpha64
            alpha128 = sb.tile([P, 1], F32, tag="alpha128")
            nc.vector.memset(alpha128[:], SLOPE)
            g["alpha128"] = alpha128
            base = sb.tile([P, NCH * 8], U32, tag="base")
            nc.gpsimd.iota(base[:], pattern=[[CS, NCH], [0, 8]], base=0,
                           channel_multiplier=0)
            g["base"] = base

            # layer inputs: 2*X^T ping-pong
            x2t_a = sb.tile([H, N], F32, tag="x2t_a")
            x2t_b = sb.tile([H, N], F32, tag="x2t_b")
            vbuf_view = sb.tile([P, NT * H], F32, tag="obuf")
            g["vbuf_view"] = vbuf_view

            # load pos -> 2*X^T (rows 0..2)
            xsb = sb.tile([P, NT * 3], F32, tag="xsb")
            nc.sync.dma_start(xsb[:].rearrange("p (t d) -> p t d", d=3),
                              pos_d[:].rearrange("(t p) d -> p t d", p=P))
            for t in range(NT):
                pt = pp_tp.tile([3, P], F32, tag="tp")
                nc.tensor.transpose(out=pt[:], in_=xsb[:, t * 3:(t + 1) * 3],
                                    identity=ident[:])
                nc.scalar.mul(out=x2t_a[0:3, t * P:(t + 1) * P], in_=pt[:], mul=2.0)

            _build_layer(nc, tc, g, 0, 3, x2t_a, x2t_b,
                         wnames["w11"], wnames["b11"], wnames["w12"], wnames["b12"],
                         wnames["w13"], wnames["b13"], v_ds[0])
            _build_layer(nc, tc, g, 1, H, x2t_b, x2t_a,
                         wnames["w21"], wnames["b21"], wnames["w22"], wnames["b22"],
                         wnames["w23"], wnames["b23"], v_ds[1])
            _build_layer(nc, tc, g, 2, H, x2t_a, None,
                         wnames["w31"], wnames["b31"], wnames["w32"], wnames["b32"],
                         wnames["w33"], wnames["b33"], v_ds[2], out_d=out_d)
    nc.finalize()
    return nc


def kernel(**inputs):
    if "nc" not in _CACHE:
        _CACHE["nc"] = build()
    nc = _CACHE["nc"]
    pos = np.ascontiguousarray(np.asarray(inputs["pos"], dtype=np.float32))
    weights = {k: np.ascontiguousarray(np.asarray(v, dtype=np.float32))
               for k, v in inputs.items() if k != "pos"}
    in_maps = []
    for b in range(B):
        m = {"pos": pos[b]}
        m.update(weights)
        in_maps.append(m)
    res = run_bass_kernel_spmd(nc, in_maps, core_ids=list(range(B)))
    out = np.stack([res.results[b]["out"] for b in range(B)], axis=0)
    return out


if __name__ == "__main__":
    rng = np.random.default_rng(0)
    fake = {"pos": rng.standard_normal((B, N, 3)).astype(np.float32)}
    for pfx in ("1", "2", "3"):
        d2 = 6 if pfx == "1" else 128
        fake[f"w{pfx}1"] = rng.standard_normal((d2, H)).astype(np.float32) * 0.2
        fake[f"w{pfx}2"] = rng.standard_normal((H, H)).astype(np.float32) * 0.12
        fake[f"w{pfx}3"] = rng.standard_normal((H, H)).astype(np.float32) * 0.12
        for j in ("1", "2", "3"):
            fake[f"b{pfx}{j}"] = np.zeros(H, np.float32)
    o = kernel(**fake)
    print("out", o.shape, o.dtype, float(np.abs(o).max()))


# revision 6
# speedup vs baseline: 12.7639x; 12.7639x over previous
"""DGCNN (3x DynamicEdgeConv, kNN=20) Trainium2 Bass kernel.

Self-contained: `kernel(**inputs) -> np.ndarray` takes the full inputs from
setup_inputs() (pos [8,4096,3] + 9 weight/bias pairs) and returns [8,4096,64].

Sharding: data-parallel over batch B=8 -> one point cloud per NeuronCore,
weights replicated. Each core runs the identical program on its slice.

Per-core, per-layer pipeline (N=4096 points, feature dim D in {3,64}, H=64):
  phase 0: X2T = 2*X^T in SBUF [64,4096]; negsq2 = -2*||x_j||^2 row;
           U = x@(W1a-W1b)+b1 (point-major, SBUF), V = x@W1b -> DRAM [4096,64]
  per row-tile t (128 points):
    S = 4 x_i.x_j - 2 sq_j  (PE, accumulating -2sq row; row-monotone == -dist)
    top-20: per-128-chunk top-8 via DVE max/max_index -> 256 candidates,
            3x (max8 + match_replace) -> 24 winners, positions via max_index,
            per-row candidate-index lookup via gpsimd indirect_copy + 16
            strided-partition diagonal DMAs -> idx [128,24] (first 20 valid)
    h1 = leaky(U_i + V_j): prefill U (ACT), 20x indirect DMA gather with
            CCE-add, Prelu(alpha=0.2)
    h1 -> 20 PE transposes -> h1T [64, 20*128] (edge k-major)
    h2T = Prelu(W2^T@h1T + b2), h3T = Prelu(W3^T@h2T + b3)   (PE + ACT)
    out tile = max over k (DVE strided reduce) -> next layer's X2T
"""
import numpy as np

import concourse.bass as bass
import concourse.bacc as bacc
import concourse.mybir as mybir
import concourse.tile as tile
from concourse.bass_utils import run_bass_kernel_spmd
from concourse.masks import make_identity

F32 = mybir.dt.float32
U16 = mybir.dt.uint16
U32 = mybir.dt.uint32
AF = mybir.ActivationFunctionType
ALU = mybir.AluOpType

B = 8
N = 4096
P = 128
NT = N // P            # 32 row tiles
CS = 128               # top-k chunk size
NCH = N // CS          # 32 chunks
K = 20
H = 64
SLOPE = 0.2
NEG = -3.0e38
NSW = 4                # SWDGE queues for the edge gathers

_CACHE = {}


def _gather_q(nc, q, **kw):
    bi = nc.gpsimd.indirect_dma_start(**kw)
    if q:
        bi.ins.queue = f"qPoolDynamic{q}"
    return bi


def _build_layer(nc, tc, g, li, d_in, x2t, x2t_next, w1_d, b1_d, w2_d, b2_d,
                 w3_d, b3_d, v_d, out_d=None):
    """Emit one EdgeConv layer. x2t holds 2*X^T (rows 0..d_in-1 valid).
    Writes 2*X_next^T into x2t_next, or the final output to out_d (layer 3)."""
    sb = g["sb"]
    pp_s, pp_tp, pp_h = g["pp_s"], g["pp_tp"], g["pp_h"]
    ident, ones1, alpha64, alpha128, base = (
        g["ident"], g["ones1"], g["alpha64"], g["alpha128"], g["base"])

    # ---- phase 0: weights ----
    wa = sb.tile([d_in, H], F32, tag="wa")
    wb = sb.tile([d_in, H], F32, tag="wb")
    nc.sync.dma_start(wa[:], w1_d[0:d_in, :])
    nc.sync.dma_start(wb[:], w1_d[d_in:2 * d_in, :])
    wd2 = sb.tile([d_in, H], F32, tag="wd2")
    nc.vector.tensor_tensor(out=wd2[:], in0=wa[:], in1=wb[:], op=ALU.subtract)
    nc.vector.tensor_scalar_mul(wd2[:], wd2[:], 0.5)
    wb2 = sb.tile([d_in, H], F32, tag="wb2")
    nc.vector.tensor_scalar_mul(wb2[:], wb[:], 0.5)
    w2 = sb.tile([H, H], F32, tag="w2")
    nc.sync.dma_start(w2[:], w2_d[:])
    w3 = sb.tile([H, H], F32, tag="w3")
    nc.sync.dma_start(w3[:], w3_d[:])
    b1r = sb.tile([1, H], F32, tag="b1r")
    nc.sync.dma_start(b1r[:], b1_d[:].unsqueeze(0))
    b2c = sb.tile([H, 1], F32, tag="b2c")
    nc.sync.dma_start(b2c[:], b2_d[:].unsqueeze(1))
    b3c = sb.tile([H, 1], F32, tag="b3c")
    nc.sync.dma_start(b3c[:], b3_d[:].unsqueeze(1))

    # ---- phase 0: negsq2 = -2*sq (from X2T: colsum(X2T^2) = 4 sq) ----
    xsq = g["s_pool"].tile([P, N], F32, tag="s")   # borrow an S buffer
    nc.scalar.activation(out=xsq[0:d_in, :], in_=x2t[0:d_in, :], func=AF.Square)
    onescol = sb.tile([d_in, 1], F32, tag="ones")
    nc.vector.memset(onescol[:], 1.0)
    negsq2 = sb.tile([1, N], F32, tag="negsq")
    for c in range(N // 512):
        ps = pp_tp.tile([1, 512], F32, tag="tp")
        nc.tensor.matmul(out=ps[:], lhsT=onescol[:], rhs=xsq[0:d_in, c * 512:(c + 1) * 512],
                         start=True, stop=True)
        nc.scalar.activation(out=negsq2[:, c * 512:(c + 1) * 512], in_=ps[:],
                             func=AF.Copy, scale=-0.5)

    # ---- phase 0: U (point-major SBUF) and V (point-major DRAM) ----
    u_sb = sb.tile([P, NT * H], F32, tag="u")
    vbuf = sb.tile([P, NT * H], F32, tag="vbuf")
    for t in range(NT):
        lhs = x2t[0:d_in, t * P:(t + 1) * P]
        pu = pp_tp.tile([P, H], F32, tag="tp")
        nc.tensor.matmul(out=pu[:], lhsT=lhs, rhs=wd2[:], start=True, stop=False)
        nc.tensor.matmul(out=pu[:], lhsT=ones1[:, 0:P], rhs=b1r[:], start=False, stop=True)
        nc.scalar.copy(out=u_sb[:, t * H:(t + 1) * H], in_=pu[:])
        pv = pp_tp.tile([P, H], F32, tag="tp")
        nc.tensor.matmul(out=pv[:], lhsT=lhs, rhs=wb2[:], start=True, stop=True)
        nc.scalar.copy(out=vbuf[:, t * H:(t + 1) * H], in_=pv[:])
    nc.sync.dma_start(v_d[:].rearrange("(t p) f -> p t f", p=P),
                      vbuf[:].rearrange("p (t f) -> p t f", f=H))

    # ---- per row-tile ----
    for t in range(NT):
        # distance tile S [128, N] (bigger = closer)
        s_sb = g["s_pool"].tile([P, N], F32, tag="s")
        for c in range(N // 512):
            ps = pp_s.tile([P, 512], F32, tag="dist")
            nc.tensor.matmul(out=ps[:], lhsT=x2t[0:d_in, t * P:(t + 1) * P],
                             rhs=x2t[0:d_in, c * 512:(c + 1) * 512],
                             start=True, stop=False)
            nc.tensor.matmul(out=ps[:], lhsT=ones1[:, 0:P],
                             rhs=negsq2[:, c * 512:(c + 1) * 512],
                             start=False, stop=True)
            nc.scalar.copy(out=s_sb[:, c * 512:(c + 1) * 512], in_=ps[:])

        # top-k stage 1: per-chunk top-8 values + indices
        cands = g["k_pool"].tile([P, NCH * 8], F32, tag="cands")
        glob = g["k_pool"].tile([P, NCH * 8], U32, tag="glob")
        if SKIP_TOPK:
            nc.vector.tensor_reduce(out=cands[:],
                                    in_=s_sb[:].rearrange("p (c e) -> p c e", c=NCH * 8),
                                    axis=mybir.AxisListType.X, op=ALU.max)
            nc.vector.tensor_copy(out=glob[:], in_=base[:])
        else:
            for c in range(NCH):
                sl = s_sb[:, c * CS:(c + 1) * CS]
                nc.vector.max(out=cands[:, c * 8:(c + 1) * 8], in_=sl)
                nc.vector.max_index(out=glob[:, c * 8:(c + 1) * 8],
                                    in_max=cands[:, c * 8:(c + 1) * 8], in_values=sl)
            nc.vector.tensor_tensor(out=glob[:], in0=glob[:], in1=base[:], op=ALU.add)

        # stage 2: global top-24 of candidates
        winners = g["k_pool"].tile([P, 24], F32, tag="win")
        pos = g["k_pool"].tile([P, 24], U16, tag="pos")
        work1 = g["k_pool"].tile([P, NCH * 8], F32, tag="wk1")
        work2 = g["k_pool"].tile([P, NCH * 8], F32, tag="wk2")
        nc.vector.max(out=winners[:, 0:8], in_=cands[:])
        nc.vector.match_replace(out=work1[:], in_to_replace=winners[:, 0:8],
                                in_values=cands[:], imm_value=NEG)
        nc.vector.max(out=winners[:, 8:16], in_=work1[:])
        nc.vector.match_replace(out=work2[:], in_to_replace=winners[:, 8:16],
                                in_values=work1[:], imm_value=NEG)
        nc.vector.max(out=winners[:, 16:24], in_=work2[:])
        for r in range(3):
            nc.vector.max_index(out=pos[:, r * 8:(r + 1) * 8],
                                in_max=winners[:, r * 8:(r + 1) * 8],
                                in_values=cands[:])

        # per-row index lookup: look[p, s*16+r] = glob[p, pos[row16q+r, s]]
        look = g["k_pool"].tile([P, 24 * 16], U32, tag="look")
        nc.gpsimd.indirect_copy(out=look[:], data=glob[:], idxs=pos[:],
                                i_know_ap_gather_is_preferred=True)
        idxt = g["k_pool"].tile([P, 24], U32, tag="idxt")
        for r in range(16):
            nc.sync.dma_start(idxt[r:P:16, 0:24], look[r:P:16, r:24 * 16:16])

        # edge features: h1 = leaky(U_i + V_j)
        h1 = g["h1_pool"].tile([P, K * H], F32, tag="h1")
        nc.scalar.copy(out=h1[:].rearrange("p (k f) -> p k f", k=K),
                       in_=u_sb[:, t * H:(t + 1) * H].unsqueeze(1).to_broadcast([P, K, H]))
        for k in (range(0) if SKIP_GATHER else range(K)):
            _gather_q(nc, k % NSW,
                      out=h1[:, k * H:(k + 1) * H], out_offset=None,
                      in_=v_d[:],
                      in_offset=bass.IndirectOffsetOnAxis(ap=idxt[:, k:k + 1], axis=0),
                      compute_op=ALU.add)
        nc.scalar.activation(out=h1[:], in_=h1[:], func=AF.Prelu, alpha=alpha128[:])

        # transpose to edge-major h1T [64, k*128+i]
        h1t = g["ht_pool"].tile([H, K * P], F32, tag="ht")
        for kc in range(5):
            pt = pp_tp.tile([H, 512], F32, tag="tp")
            for j in range(4):
                k = kc * 4 + j
                nc.tensor.transpose(out=pt[:, j * P:(j + 1) * P],
                                    in_=h1[:, k * H:(k + 1) * H], identity=ident[:])
            nc.scalar.copy(out=h1t[:, kc * 512:(kc + 1) * 512], in_=pt[:])

        # MLP layers 2, 3 (feature-major, edges on the free axis)
        h2t = g["ht_pool"].tile([H, K * P], F32, tag="ht")
        for e in range(5):
            ph = pp_h.tile([H, 512], F32, tag="h")
            nc.tensor.matmul(out=ph[:], lhsT=w2[:], rhs=h1t[:, e * 512:(e + 1) * 512],
                             start=True, stop=True)
            nc.scalar.activation(out=h2t[:, e * 512:(e + 1) * 512], in_=ph[:],
                                 func=AF.Prelu, bias=b2c[:], alpha=alpha64[:])
        h3t = g["ht_pool"].tile([H, K * P], F32, tag="ht")
        for e in range(5):
            ph = pp_h.tile([H, 512], F32, tag="h")
            nc.tensor.matmul(out=ph[:], lhsT=w3[:], rhs=h2t[:, e * 512:(e + 1) * 512],
                             start=True, stop=True)
            nc.scalar.activation(out=h3t[:, e * 512:(e + 1) * 512], in_=ph[:],
                                 func=AF.Prelu, bias=b3c[:], alpha=alpha64[:])

        # aggregate: max over k (innermost stride-128 axis)
        ftile = g["f_pool"].tile([H, P], F32, tag="f")
        nc.vector.tensor_reduce(out=ftile[:],
                                in_=h3t[:].rearrange("h (k i) -> h i k", k=K),
                                axis=mybir.AxisListType.X, op=ALU.max)
        if out_d is None:
            nc.scalar.mul(out=x2t_next[0:H, t * P:(t + 1) * P], in_=ftile[:], mul=2.0)
        else:
            po = pp_tp.tile([P, H], F32, tag="tp")
            nc.tensor.transpose(out=po[:], in_=ftile[:], identity=ident[0:H, 0:H])
            nc.scalar.copy(out=g["vbuf_view"][:, t * H:(t + 1) * H], in_=po[:])
    if out_d is not None:
        nc.sync.dma_start(out_d[:].rearrange("(t p) f -> p t f", p=P),
                          g["vbuf_view"][:].rearrange("p (t f) -> p t f", f=H))


def build():
    nc = bacc.Bacc("TRN2", target_bir_lowering=False, debug=False,
                   num_swdge_queues=NSW)
    pos_d = nc.dram_tensor("pos", [N, 3], F32, kind="ExternalInput")
    wnames = {}
    for li, pfx in enumerate(("1", "2", "3")):
        d2 = 6 if li == 0 else 128
        wnames[f"w{pfx}1"] = nc.dram_tensor(f"w{pfx}1", [d2, H], F32, kind="ExternalInput")
        wnames[f"w{pfx}2"] = nc.dram_tensor(f"w{pfx}2", [H, H], F32, kind="ExternalInput")
        wnames[f"w{pfx}3"] = nc.dram_tensor(f"w{pfx}3", [H, H], F32, kind="ExternalInput")
        for j in ("1", "2", "3"):
            wnames[f"b{pfx}{j}"] = nc.dram_tensor(f"b{pfx}{j}", [H], F32, kind="ExternalInput")
    out_d = nc.dram_tensor("out", [N, H], F32, kind="ExternalOutput")
    v_ds = [nc.dram_tensor(f"vtab{li}", [N, H], F32) for li in range(3)]

    with tile.TileContext(nc) as tc:
        with tc.tile_pool(name="sb", bufs=1) as sb, \
             tc.tile_pool(name="s_pool", bufs=2) as s_pool, \
             tc.tile_pool(name="k_pool", bufs=2) as k_pool, \
             tc.tile_pool(name="h1_pool", bufs=2) as h1_pool, \
             tc.tile_pool(name="ht_pool", bufs=4) as ht_pool, \
             tc.tile_pool(name="f_pool", bufs=2) as f_pool, \
             tc.tile_pool(name="pp_s", bufs=3, space="PSUM") as pp_s, \
             tc.tile_pool(name="pp_tp", bufs=2, space="PSUM") as pp_tp, \
             tc.tile_pool(name="pp_h", bufs=2, space="PSUM") as pp_h:

            g = dict(sb=sb, s_pool=s_pool, k_pool=k_pool, h1_pool=h1_pool,
                     ht_pool=ht_pool, f_pool=f_pool,
                     pp_s=pp_s, pp_tp=pp_tp, pp_h=pp_h)

            ident = sb.tile([P, P], F32, tag="ident")
            make_identity(nc, ident)
            g["ident"] = ident
            ones1 = sb.tile([1, P], F32, tag="ones1")
            nc.vector.memset(ones1[:], 1.0)
            g["ones1"] = ones1
            alpha64 = sb.tile([H, 1], F32, tag="alpha64")
            nc.vector.memset(alpha64[:], SLOPE)
            g["alpha64"] = alpha64
            alpha128 = sb.tile([P, 1], F32, tag="alpha128")
            nc.vector.memset(alpha128[:], SLOPE)
            g["alpha128"] = alpha128
            base = sb.tile([P, NCH * 8], U32, tag="base")
            nc.gpsimd.iota(base[:], pattern=[[CS, NCH], [0, 8]], base=0,
                           channel_multiplier=0)
            g["base"] = base

            # layer inputs: 2*X^T ping-pong
            x2t_a = sb.tile([H, N], F32, tag="x2t_a")
            x2t_b = sb.tile([H, N], F32, tag="x2t_b")
            vbuf_view = sb.tile([P, NT * H], F32, tag="obuf")
            g["vbuf_view"] = vbuf_view

            # load pos -> 2*X^T (rows 0..2)
            xsb = sb.tile([P, NT * 3], F32, tag="xsb")
            nc.sync.dma_start(xsb[:].rearrange("p (t d) -> p t d", d=3),
                              pos_d[:].rearrange("(t p) d -> p t d", p=P))
            for t in range(NT):
                pt = pp_tp.tile([3, P], F32, tag="tp")
                nc.tensor.transpose(out=pt[:], in_=xsb[:, t * 3:(t + 1) * 3],
                                    identity=ident[:])
                nc.scalar.mul(out=x2t_a[0:3, t * P:(t + 1) * P], in_=pt[:], mul=2.0)

            _build_layer(nc, tc, g, 0, 3, x2t_a, x2t_b,
                         wnames["w11"], wnames["b11"], wnames["w12"], wnames["b12"],
                         wnames["w13"], wnames["b13"], v_ds[0])
            _build_layer(nc, tc, g, 1, H, x2t_b, x2t_a,
                         wnames["w21"], wnames["b21"], wnames["w22"], wnames["b22"],
                         wnames["w23"], wnames["b23"], v_ds[1])
            _build_layer(nc, tc, g, 2, H, x2t_a, None,
                         wnames["w31"], wnames["b31"], wnames["w32"], wnames["b32"],
                         wnames["w33"], wnames["b33"], v_ds[2], out_d=out_d)
    nc.finalize()
    return nc


def kernel(**inputs):
    if "nc" not in _CACHE:
        _CACHE["nc"] = build()
    nc = _CACHE["nc"]
    pos = np.ascontiguousarray(np.asarray(inputs["pos"], dtype=np.float32))
    weights = {k: np.ascontiguousarray(np.asarray(v, dtype=np.float32))
               for k, v in inputs.items() if k != "pos"}
    in_maps = []
    for b in range(B):
        m = {"pos": pos[b]}
        m.update(weights)
        in_maps.append(m)
    res = run_bass_kernel_spmd(nc, in_maps, core_ids=list(range(B)))
    out = np.stack([res.results[b]["out"] for b in range(B)], axis=0)
    return out


if __name__ == "__main__":
    rng = np.random.default_rng(0)
    fake = {"pos": rng.standard_normal((B, N, 3)).astype(np.float32)}
    for pfx in ("1", "2", "3"):
        d2 = 6 if pfx == "1" else 128
        fake[f"w{pfx}1"] = rng.standard_normal((d2, H)).astype(np.float32) * 0.2
        fake[f"w{pfx}2"] = rng.standard_normal((H, H)).astype(np.float32) * 0.12
        fake[f"w{pfx}3"] = rng.standard_normal((H, H)).astype(np.float32) * 0.12
        for j in ("1", "2", "3"):
            fake[f"b{pfx}{j}"] = np.zeros(H, np.float32)
    o = kernel(**fake)
    print("out", o.shape, o.dtype, float(np.abs(o).max()))


# revision 7
# speedup vs baseline: 13.3260x; 1.0440x over previous
"""DGCNN (3x DynamicEdgeConv, kNN=20) Trainium2 Bass kernel.

Self-contained: `kernel(**inputs) -> np.ndarray` takes the full inputs from
setup_inputs() (pos [8,4096,3] + 9 weight/bias pairs) and returns [8,4096,64].

Sharding: data-parallel over batch B=8 -> one point cloud per NeuronCore,
weights replicated. Each core runs the identical program on its slice.

Per-core, per-layer pipeline (N=4096 points, feature dim D in {3,64}, H=64):
  phase 0: X2T = 2*X^T in SBUF [64,4096]; negsq2 = -2*||x_j||^2 row;
           U = x@(W1a-W1b)+b1 (point-major, SBUF), V = x@W1b -> DRAM [4096,64]
  per row-tile t (128 points):
    S = 4 x_i.x_j - 2 sq_j  (PE, accumulating -2sq row; row-monotone == -dist)
    top-20: per-128-chunk top-8 via DVE max/max_index -> 256 candidates,
            3x (max8 + match_replace) -> 24 winners, positions via max_index,
            per-row candidate-index lookup via gpsimd indirect_copy + 16
            strided-partition diagonal DMAs -> idx [128,24] (first 20 valid)
    h1 = leaky(U_i + V_j): prefill U (ACT), 20x indirect DMA gather with
            CCE-add, Prelu(alpha=0.2)
    h1 -> 20 PE transposes -> h1T [64, 20*128] (edge k-major)
    h2T = Prelu(W2^T@h1T + b2), h3T = Prelu(W3^T@h2T + b3)   (PE + ACT)
    out tile = max over k (DVE strided reduce) -> next layer's X2T
"""
import numpy as np

import concourse.bass as bass
import concourse.bacc as bacc
import concourse.mybir as mybir
import concourse.tile as tile
from concourse.bass_utils import run_bass_kernel_spmd
from concourse.masks import make_identity

F32 = mybir.dt.float32
U16 = mybir.dt.uint16
U32 = mybir.dt.uint32
AF = mybir.ActivationFunctionType
ALU = mybir.AluOpType

B = 8
N = 4096
P = 128
NT = N // P            # 32 row tiles
CS = 128               # top-k chunk size
NCH = N // CS          # 32 chunks
K = 20
H = 64
SLOPE = 0.2
NEG = -3.0e38
NSW = 4                # SWDGE queues for the edge gathers

_CACHE = {}


def _gather_q(nc, q, **kw):
    bi = nc.gpsimd.indirect_dma_start(**kw)
    if q:
        bi.ins.queue = f"qPoolDynamic{q}"
    return bi


def _build_layer(nc, tc, g, li, d_in, x2t, x2t_next, w1_d, b1_d, w2_d, b2_d,
                 w3_d, b3_d, v_d, out_d=None):
    """Emit one EdgeConv layer. x2t holds 2*X^T (rows 0..d_in-1 valid).
    Writes 2*X_next^T into x2t_next, or the final output to out_d (layer 3)."""
    sb = g["sb"]
    pp_s, pp_tp, pp_h = g["pp_s"], g["pp_tp"], g["pp_h"]
    ident, ones1, alpha64, alpha128, base = (
        g["ident"], g["ones1"], g["alpha64"], g["alpha128"], g["base"])

    # ---- phase 0: weights ----
    wa = sb.tile([d_in, H], F32, tag="wa")
    wb = sb.tile([d_in, H], F32, tag="wb")
    nc.sync.dma_start(wa[:], w1_d[0:d_in, :])
    nc.sync.dma_start(wb[:], w1_d[d_in:2 * d_in, :])
    wd2 = sb.tile([d_in, H], F32, tag="wd2")
    nc.vector.tensor_tensor(out=wd2[:], in0=wa[:], in1=wb[:], op=ALU.subtract)
    nc.vector.tensor_scalar_mul(wd2[:], wd2[:], 0.5)
    wb2 = sb.tile([d_in, H], F32, tag="wb2")
    nc.vector.tensor_scalar_mul(wb2[:], wb[:], 0.5)
    w2 = sb.tile([H, H], F32, tag="w2")
    nc.sync.dma_start(w2[:], w2_d[:])
    w3 = sb.tile([H, H], F32, tag="w3")
    nc.sync.dma_start(w3[:], w3_d[:])
    b1r = sb.tile([1, H], F32, tag="b1r")
    nc.sync.dma_start(b1r[:], b1_d[:].unsqueeze(0))
    b2c = sb.tile([H, 1], F32, tag="b2c")
    nc.sync.dma_start(b2c[:], b2_d[:].unsqueeze(1))
    b3c = sb.tile([H, 1], F32, tag="b3c")
    nc.sync.dma_start(b3c[:], b3_d[:].unsqueeze(1))

    # ---- phase 0: negsq2 = -2*sq (from X2T: colsum(X2T^2) = 4 sq) ----
    xsq = g["s_pool"].tile([P, N], F32, tag="s")   # borrow an S buffer
    nc.scalar.activation(out=xsq[0:d_in, :], in_=x2t[0:d_in, :], func=AF.Square)
    onescol = sb.tile([d_in, 1], F32, tag="ones")
    nc.vector.memset(onescol[:], 1.0)
    negsq2 = sb.tile([1, N], F32, tag="negsq")
    for c in range(N // 512):
        ps = pp_tp.tile([1, 512], F32, tag="tp")
        nc.tensor.matmul(out=ps[:], lhsT=onescol[:], rhs=xsq[0:d_in, c * 512:(c + 1) * 512],
                         start=True, stop=True)
        nc.scalar.activation(out=negsq2[:, c * 512:(c + 1) * 512], in_=ps[:],
                             func=AF.Copy, scale=-0.5)

    # ---- phase 0: U (point-major SBUF) and V (point-major DRAM) ----
    u_sb = sb.tile([P, NT * H], F32, tag="u")
    vbuf = sb.tile([P, NT * H], F32, tag="vbuf")
    for t in range(NT):
        lhs = x2t[0:d_in, t * P:(t + 1) * P]
        pu = pp_tp.tile([P, H], F32, tag="tp")
        nc.tensor.matmul(out=pu[:], lhsT=lhs, rhs=wd2[:], start=True, stop=False)
        nc.tensor.matmul(out=pu[:], lhsT=ones1[:, 0:P], rhs=b1r[:], start=False, stop=True)
        nc.scalar.copy(out=u_sb[:, t * H:(t + 1) * H], in_=pu[:])
        pv = pp_tp.tile([P, H], F32, tag="tp")
        nc.tensor.matmul(out=pv[:], lhsT=lhs, rhs=wb2[:], start=True, stop=True)
        nc.scalar.copy(out=vbuf[:, t * H:(t + 1) * H], in_=pv[:])
    nc.sync.dma_start(v_d[:].rearrange("(t p) f -> p t f", p=P),
                      vbuf[:].rearrange("p (t f) -> p t f", f=H))

    # ---- per row-tile ----
    for t in range(NT):
        # distance tile S [128, N] (bigger = closer)
        s_sb = g["s_pool"].tile([P, N], F32, tag="s")
        for c in range(N // 512):
            ps = pp_s.tile([P, 512], F32, tag="dist")
            nc.tensor.matmul(out=ps[:], lhsT=x2t[0:d_in, t * P:(t + 1) * P],
                             rhs=x2t[0:d_in, c * 512:(c + 1) * 512],
                             start=True, stop=False)
            nc.tensor.matmul(out=ps[:], lhsT=ones1[:, 0:P],
                             rhs=negsq2[:, c * 512:(c + 1) * 512],
                             start=False, stop=True)
            nc.scalar.copy(out=s_sb[:, c * 512:(c + 1) * 512], in_=ps[:])

        # top-k stage 1: per-chunk top-8 values (exact cover: max chunk load 6)
        cands = g["k_pool"].tile([P, NCH * 8], F32, tag="cands")
        for c in range(NCH):
            nc.vector.max(out=cands[:, c * 8:(c + 1) * 8],
                          in_=s_sb[:, c * CS:(c + 1) * CS])

        # stage 2: global top-24 of candidates
        winners = g["k_pool"].tile([P, 24], F32, tag="win")
        work1 = g["k_pool"].tile([P, NCH * 8], F32, tag="wk1")
        work2 = g["k_pool"].tile([P, NCH * 8], F32, tag="wk2")
        nc.vector.max(out=winners[:, 0:8], in_=cands[:])
        nc.vector.match_replace(out=work1[:], in_to_replace=winners[:, 0:8],
                                in_values=cands[:], imm_value=NEG)
        nc.vector.max(out=winners[:, 8:16], in_=work1[:])
        nc.vector.match_replace(out=work2[:], in_to_replace=winners[:, 8:16],
                                in_values=work1[:], imm_value=NEG)
        nc.vector.max(out=winners[:, 16:24], in_=work2[:])

        # global indices: full-width max_index on the original S rows
        idxt = g["k_pool"].tile([P, 24], U32, tag="idxt")
        for r in range(3):
            nc.vector.max_index(out=idxt[:, r * 8:(r + 1) * 8],
                                in_max=winners[:, r * 8:(r + 1) * 8],
                                in_values=s_sb[:])

        # edge features: h1 = leaky(U_i + V_j)
        h1 = g["h1_pool"].tile([P, K * H], F32, tag="h1")
        nc.scalar.copy(out=h1[:].rearrange("p (k f) -> p k f", k=K),
                       in_=u_sb[:, t * H:(t + 1) * H].unsqueeze(1).to_broadcast([P, K, H]))
        for k in (range(0) if SKIP_GATHER else range(K)):
            _gather_q(nc, k % NSW,
                      out=h1[:, k * H:(k + 1) * H], out_offset=None,
                      in_=v_d[:],
                      in_offset=bass.IndirectOffsetOnAxis(ap=idxt[:, k:k + 1], axis=0),
                      compute_op=ALU.add)
        nc.scalar.activation(out=h1[:], in_=h1[:], func=AF.Prelu, alpha=alpha128[:])

        # transpose to edge-major h1T [64, k*128+i]
        h1t = g["ht_pool"].tile([H, K * P], F32, tag="ht")
        for kc in range(5):
            pt = pp_tp.tile([H, 512], F32, tag="tp")
            for j in range(4):
                k = kc * 4 + j
                nc.tensor.transpose(out=pt[:, j * P:(j + 1) * P],
                                    in_=h1[:, k * H:(k + 1) * H], identity=ident[:])
            nc.scalar.copy(out=h1t[:, kc * 512:(kc + 1) * 512], in_=pt[:])

        # MLP layers 2, 3 (feature-major, edges on the free axis)
        h2t = g["ht_pool"].tile([H, K * P], F32, tag="ht")
        for e in range(5):
            ph = pp_h.tile([H, 512], F32, tag="h")
            nc.tensor.matmul(out=ph[:], lhsT=w2[:], rhs=h1t[:, e * 512:(e + 1) * 512],
                             start=True, stop=True)
            nc.scalar.activation(out=h2t[:, e * 512:(e + 1) * 512], in_=ph[:],
                                 func=AF.Prelu, bias=b2c[:], alpha=alpha64[:])
        h3t = g["ht_pool"].tile([H, K * P], F32, tag="ht")
        for e in range(5):
            ph = pp_h.tile([H, 512], F32, tag="h")
            nc.tensor.matmul(out=ph[:], lhsT=w3[:], rhs=h2t[:, e * 512:(e + 1) * 512],
                             start=True, stop=True)
            nc.scalar.activation(out=h3t[:, e * 512:(e + 1) * 512], in_=ph[:],
                                 func=AF.Prelu, bias=b3c[:], alpha=alpha64[:])

        # aggregate: max over k (innermost stride-128 axis)
        ftile = g["f_pool"].tile([H, P], F32, tag="f")
        nc.vector.tensor_reduce(out=ftile[:],
                                in_=h3t[:].rearrange("h (k i) -> h i k", k=K),
                                axis=mybir.AxisListType.X, op=ALU.max)
        if out_d is None:
            nc.scalar.mul(out=x2t_next[0:H, t * P:(t + 1) * P], in_=ftile[:], mul=2.0)
        else:
            po = pp_tp.tile([P, H], F32, tag="tp")
            nc.tensor.transpose(out=po[:], in_=ftile[:], identity=ident[0:H, 0:H])
            nc.scalar.copy(out=g["vbuf_view"][:, t * H:(t + 1) * H], in_=po[:])
    if out_d is not None:
        nc.sync.dma_start(out_d[:].rearrange("(t p) f -> p t f", p=P),
                          g["vbuf_view"][:].rearrange("p (t f) -> p t f", f=H))


def build():
    nc = bacc.Bacc("TRN2", target_bir_lowering=False, debug=False,
                   num_swdge_queues=NSW)
    pos_d = nc.dram_tensor("pos", [N, 3], F32, kind="ExternalInput")
    wnames = {}
    for li, pfx in enumerate(("1", "2", "3")):
        d2 = 6 if li == 0 else 128
        wnames[f"w{pfx}1"] = nc.dram_tensor(f"w{pfx}1", [d2, H], F32, kind="ExternalInput")
        wnames[f"w{pfx}2"] = nc.dram_tensor(f"w{pfx}2", [H, H], F32, kind="ExternalInput")
        wnames[f"w{pfx}3"] = nc.dram_tensor(f"w{pfx}3", [H, H], F32, kind="ExternalInput")
        for j in ("1", "2", "3"):
            wnames[f"b{pfx}{j}"] = nc.dram_tensor(f"b{pfx}{j}", [H], F32, kind="ExternalInput")
    out_d = nc.dram_tensor("out", [N, H], F32, kind="ExternalOutput")
    v_ds = [nc.dram_tensor(f"vtab{li}", [N, H], F32) for li in range(3)]

    with tile.TileContext(nc) as tc:
        with tc.tile_pool(name="sb", bufs=1) as sb, \
             tc.tile_pool(name="s_pool", bufs=2) as s_pool, \
             tc.tile_pool(name="k_pool", bufs=2) as k_pool, \
             tc.tile_pool(name="h1_pool", bufs=2) as h1_pool, \
             tc.tile_pool(name="ht_pool", bufs=4) as ht_pool, \
             tc.tile_pool(name="f_pool", bufs=2) as f_pool, \
             tc.tile_pool(name="pp_s", bufs=3, space="PSUM") as pp_s, \
             tc.tile_pool(name="pp_tp", bufs=2, space="PSUM") as pp_tp, \
             tc.tile_pool(name="pp_h", bufs=2, space="PSUM") as pp_h:

            g = dict(sb=sb, s_pool=s_pool, k_pool=k_pool, h1_pool=h1_pool,
                     ht_pool=ht_pool, f_pool=f_pool,
                     pp_s=pp_s, pp_tp=pp_tp, pp_h=pp_h)

            ident = sb.tile([P, P], F32, tag="ident")
            make_identity(nc, ident)
            g["ident"] = ident
            ones1 = sb.tile([1, P], F32, tag="ones1")
            nc.vector.memset(ones1[:], 1.0)
            g["ones1"] = ones1
            alpha64 = sb.tile([H, 1], F32, tag="alpha64")
            nc.vector.memset(alpha64[:], SLOPE)
            g["alpha64"] = alpha64
            alpha128 = sb.tile([P, 1], F32, tag="alpha128")
            nc.vector.memset(alpha128[:], SLOPE)
            g["alpha128"] = alpha128
            base = sb.tile([P, NCH * 8], U32, tag="base")
            nc.gpsimd.iota(base[:], pattern=[[CS, NCH], [0, 8]], base=0,
                           channel_multiplier=0)
            g["base"] = base

            # layer inputs: 2*X^T ping-pong
            x2t_a = sb.tile([H, N], F32, tag="x2t_a")
            x2t_b = sb.tile([H, N], F32, tag="x2t_b")
            vbuf_view = sb.tile([P, NT * H], F32, tag="obuf")
            g["vbuf_view"] = vbuf_view

            # load pos -> 2*X^T (rows 0..2)
            xsb = sb.tile([P, NT * 3], F32, tag="xsb")
            nc.sync.dma_start(xsb[:].rearrange("p (t d) -> p t d", d=3),
                              pos_d[:].rearrange("(t p) d -> p t d", p=P))
            for t in range(NT):
                pt = pp_tp.tile([3, P], F32, tag="tp")
                nc.tensor.transpose(out=pt[:], in_=xsb[:, t * 3:(t + 1) * 3],
                                    identity=ident[:])
                nc.scalar.mul(out=x2t_a[0:3, t * P:(t + 1) * P], in_=pt[:], mul=2.0)

            _build_layer(nc, tc, g, 0, 3, x2t_a, x2t_b,
                         wnames["w11"], wnames["b11"], wnames["w12"], wnames["b12"],
                         wnames["w13"], wnames["b13"], v_ds[0])
            _build_layer(nc, tc, g, 1, H, x2t_b, x2t_a,
                         wnames["w21"], wnames["b21"], wnames["w22"], wnames["b22"],
                         wnames["w23"], wnames["b23"], v_ds[1])
            _build_layer(nc, tc, g, 2, H, x2t_a, None,
                         wnames["w31"], wnames["b31"], wnames["w32"], wnames["b32"],
                         wnames["w33"], wnames["b33"], v_ds[2], out_d=out_d)
    nc.finalize()
    return nc


def kernel(**inputs):
    if "nc" not in _CACHE:
        _CACHE["nc"] = build()
    nc = _CACHE["nc"]
    pos = np.ascontiguousarray(np.asarray(inputs["pos"], dtype=np.float32))
    weights = {k: np.ascontiguousarray(np.asarray(v, dtype=np.float32))
               for k, v in inputs.items() if k != "pos"}
    in_maps = []
    for b in range(B):
        m = {"pos": pos[b]}
        m.update(weights)
        in_maps.append(m)
    res = run_bass_kernel_spmd(nc, in_maps, core_ids=list(range(B)))
    out = np.stack([res.results[b]["out"] for b in range(B)], axis=0)
    return out


if __name__ == "__main__":
    rng = np.random.default_rng(0)
    fake = {"pos": rng.standard_normal((B, N, 3)).astype(np.float32)}
    for pfx in ("1", "2", "3"):
        d2 = 6 if pfx == "1" else 128
        fake[f"w{pfx}1"] = rng.standard_normal((d2, H)).astype(np.float32) * 0.2
        fake[f"w{pfx}2"] = rng.standard_normal((H, H)).astype(np.float32) * 0.12
        fake[f"w{pfx}3"] = rng.standard_normal((H, H)).astype(np.float32) * 0.12
        for j in ("1", "2", "3"):
            fake[f"b{pfx}{j}"] = np.zeros(H, np.float32)
    o = kernel(**fake)
    print("out", o.shape, o.dtype, float(np.abs(o).max()))


# revision 10
# speedup vs baseline: 15.4435x; 1.1589x over previous
"""DGCNN (3x DynamicEdgeConv, kNN=20) Trainium2 Bass kernel.

Self-contained: `kernel(**inputs) -> np.ndarray` takes the full inputs from
setup_inputs() (pos [8,4096,3] + 9 weight/bias pairs) and returns [8,4096,64].

Sharding: data-parallel over batch B=8 -> one point cloud per NeuronCore,
weights replicated. Each core runs the identical program on its slice.

Per-core, per-layer pipeline (N=4096 points, feature dim D in {3,64}, H=64):
  phase 0: X2T = 2*X^T in SBUF [64,4096]; negsq2 = -2*||x_j||^2 row;
           U = x@(W1a-W1b)+b1 (point-major, SBUF), V = x@W1b -> DRAM [4096,64]
  per row-tile t (128 points):
    S = 4 x_i.x_j - 2 sq_j  (PE, accumulating -2sq row; row-monotone == -dist)
    top-20: per-128-chunk top-8 via DVE max/max_index -> 256 candidates,
            3x (max8 + match_replace) -> 24 winners, positions via max_index,
            per-row candidate-index lookup via gpsimd indirect_copy + 16
            strided-partition diagonal DMAs -> idx [128,24] (first 20 valid)
    h1 = leaky(U_i + V_j): prefill U (ACT), 20x indirect DMA gather with
            CCE-add, Prelu(alpha=0.2)
    h1 -> 20 PE transposes -> h1T [64, 20*128] (edge k-major)
    h2T = Prelu(W2^T@h1T + b2), h3T = Prelu(W3^T@h2T + b3)   (PE + ACT)
    out tile = max over k (DVE strided reduce) -> next layer's X2T
"""
import numpy as np

import concourse.bass as bass
import concourse.bacc as bacc
import concourse.mybir as mybir
import concourse.tile as tile
from concourse.bass_utils import run_bass_kernel_spmd
from concourse.masks import make_identity

F32 = mybir.dt.float32
U16 = mybir.dt.uint16
U32 = mybir.dt.uint32
AF = mybir.ActivationFunctionType
ALU = mybir.AluOpType

B = 8
N = 4096
P = 128
NT = N // P            # 32 row tiles
CS = 128               # top-k chunk size
NCH = N // CS          # 32 chunks
K = 20
H = 64
SLOPE = 0.2
NEG = -3.0e38
NSW = 4                # SWDGE queues for the edge gathers

_CACHE = {}


def _gather_q(nc, q, **kw):
    bi = nc.gpsimd.indirect_dma_start(**kw)
    if q:
        bi.ins.queue = f"qPoolDynamic{q}"
    return bi


def _build_layer(nc, tc, g, li, d_in, x2t, x2t_next, w1_d, b1_d, w2_d, b2_d,
                 w3_d, b3_d, v_d, out_d=None):
    """Emit one EdgeConv layer. x2t holds 2*X^T (rows 0..d_in-1 valid).
    Writes 2*X_next^T into x2t_next, or the final output to out_d (layer 3)."""
    sb = g["sb"]
    pp_s, pp_tp, pp_h = g["pp_s"], g["pp_tp"], g["pp_h"]
    ident, ones1, alpha64, alpha128, base = (
        g["ident"], g["ones1"], g["alpha64"], g["alpha128"], g["base"])

    # ---- phase 0: weights ----
    wa = sb.tile([d_in, H], F32, tag="wa")
    wb = sb.tile([d_in, H], F32, tag="wb")
    nc.sync.dma_start(wa[:], w1_d[0:d_in, :])
    nc.sync.dma_start(wb[:], w1_d[d_in:2 * d_in, :])
    wd2 = sb.tile([d_in, H], F32, tag="wd2")
    nc.vector.tensor_tensor(out=wd2[:], in0=wa[:], in1=wb[:], op=ALU.subtract)
    nc.vector.tensor_scalar_mul(wd2[:], wd2[:], 0.5)
    wb2 = sb.tile([d_in, H], F32, tag="wb2")
    nc.vector.tensor_scalar_mul(wb2[:], wb[:], 0.5)
    w2 = sb.tile([H, H], F32, tag="w2")
    nc.sync.dma_start(w2[:], w2_d[:])
    w3 = sb.tile([H, H], F32, tag="w3")
    nc.sync.dma_start(w3[:], w3_d[:])
    b1r = sb.tile([1, H], F32, tag="b1r")
    nc.sync.dma_start(b1r[:], b1_d[:].unsqueeze(0))
    b2c = sb.tile([H, 1], F32, tag="b2c")
    nc.sync.dma_start(b2c[:], b2_d[:].unsqueeze(1))
    b3c = sb.tile([H, 1], F32, tag="b3c")
    nc.sync.dma_start(b3c[:], b3_d[:].unsqueeze(1))

    # ---- phase 0: negsq2 = -2*sq (from X2T: colsum(X2T^2) = 4 sq) ----
    xsq = g["s_pool"].tile([P, N], F32, tag="s")   # borrow an S buffer
    nc.scalar.activation(out=xsq[0:d_in, :], in_=x2t[0:d_in, :], func=AF.Square)
    onescol = sb.tile([d_in, 1], F32, tag="ones")
    nc.vector.memset(onescol[:], 1.0)
    negsq2 = sb.tile([1, N], F32, tag="negsq")
    for c in range(N // 512):
        ps = pp_tp.tile([1, 512], F32, tag="tp")
        nc.tensor.matmul(out=ps[:], lhsT=onescol[:], rhs=xsq[0:d_in, c * 512:(c + 1) * 512],
                         start=True, stop=True)
        nc.scalar.activation(out=negsq2[:, c * 512:(c + 1) * 512], in_=ps[:],
                             func=AF.Copy, scale=-0.5)
    # replicate -2sq across partitions once per layer (PE rank-1), so the
    # per-tile correction is a single wide GPSIMD add instead of 8 K=1 matmuls
    negsq_rep = sb.tile([P, N], F32, tag="negsqrep")
    for c in range(N // 512):
        ps = pp_tp.tile([P, 512], F32, tag="tp")
        nc.tensor.matmul(out=ps[:], lhsT=ones1[:, 0:P],
                         rhs=negsq2[:, c * 512:(c + 1) * 512], start=True, stop=True)
        nc.scalar.copy(out=negsq_rep[:, c * 512:(c + 1) * 512], in_=ps[:])

    # ---- phase 0: U (point-major SBUF) and V (point-major DRAM) ----
    u_sb = sb.tile([P, NT * H], F32, tag="u")
    vbuf = sb.tile([P, NT * H], F32, tag="vbuf")
    for t in range(NT):
        lhs = x2t[0:d_in, t * P:(t + 1) * P]
        pu = pp_tp.tile([P, H], F32, tag="tp")
        nc.tensor.matmul(out=pu[:], lhsT=lhs, rhs=wd2[:], start=True, stop=False)
        nc.tensor.matmul(out=pu[:], lhsT=ones1[:, 0:P], rhs=b1r[:], start=False, stop=True)
        nc.scalar.copy(out=u_sb[:, t * H:(t + 1) * H], in_=pu[:])
        pv = pp_tp.tile([P, H], F32, tag="tp")
        nc.tensor.matmul(out=pv[:], lhsT=lhs, rhs=wb2[:], start=True, stop=True)
        nc.scalar.copy(out=vbuf[:, t * H:(t + 1) * H], in_=pv[:])
    nc.sync.dma_start(v_d[:].rearrange("(t p) f -> p t f", p=P),
                      vbuf[:].rearrange("p (t f) -> p t f", f=H))

    # ---- per row-tile ----
    for t in range(NT):
        # distance tile S [128, N] (bigger = closer)
        s_sb = g["s_pool"].tile([P, N], F32, tag="s")
        for c in range(N // 512):
            ps = pp_s.tile([P, 512], F32, tag="dist")
            nc.tensor.matmul(out=ps[:], lhsT=x2t[0:d_in, t * P:(t + 1) * P],
                             rhs=x2t[0:d_in, c * 512:(c + 1) * 512],
                             start=True, stop=True)
            nc.scalar.copy(out=s_sb[:, c * 512:(c + 1) * 512], in_=ps[:])
        nc.gpsimd.tensor_tensor(out=s_sb[:], in0=s_sb[:], in1=negsq_rep[:], op=ALU.add)

        # exact top-24 of each row with in-place knockout (8 wide DVE insts):
        # max8 -> indices -> replace-with-NEG, three rounds. match_replace
        # only zaps already-extracted values, so later max_index positions
        # in the modified S equal positions in the original.
        winners = g["k_pool"].tile([P, 24], F32, tag="win")
        idxt = g["k_pool"].tile([P, 24], U32, tag="idxt")
        for r in range(3):
            nc.vector.max(out=winners[:, r * 8:(r + 1) * 8], in_=s_sb[:])
            nc.vector.max_index(out=idxt[:, r * 8:(r + 1) * 8],
                                in_max=winners[:, r * 8:(r + 1) * 8],
                                in_values=s_sb[:])
            if r < 2:
                nc.vector.match_replace(out=s_sb[:],
                                        in_to_replace=winners[:, r * 8:(r + 1) * 8],
                                        in_values=s_sb[:], imm_value=NEG)

        # edge features: h1 = leaky(U_i + V_j)
        h1 = g["h1_pool"].tile([P, K * H], F32, tag="h1")
        nc.scalar.copy(out=h1[:].rearrange("p (k f) -> p k f", k=K),
                       in_=u_sb[:, t * H:(t + 1) * H].unsqueeze(1).to_broadcast([P, K, H]))
        for k in (range(0) if SKIP_GATHER else range(K)):
            _gather_q(nc, k % NSW,
                      out=h1[:, k * H:(k + 1) * H], out_offset=None,
                      in_=v_d[:],
                      in_offset=bass.IndirectOffsetOnAxis(ap=idxt[:, k:k + 1], axis=0),
                      compute_op=ALU.add)
        nc.scalar.activation(out=h1[:], in_=h1[:], func=AF.Prelu, alpha=alpha128[:])

        # transpose to edge-major h1T [64, k*128+i]
        h1t = g["ht_pool"].tile([H, K * P], F32, tag="ht")
        for kc in range(5):
            pt = pp_tp.tile([H, 512], F32, tag="tp")
            for j in range(4):
                k = kc * 4 + j
                nc.tensor.transpose(out=pt[:, j * P:(j + 1) * P],
                                    in_=h1[:, k * H:(k + 1) * H], identity=ident[:])
            nc.scalar.copy(out=h1t[:, kc * 512:(kc + 1) * 512], in_=pt[:])

        # MLP layers 2, 3 (feature-major, edges on the free axis)
        h2t = g["ht_pool"].tile([H, K * P], F32, tag="ht")
        for e in range(5):
            ph = pp_h.tile([H, 512], F32, tag="h")
            nc.tensor.matmul(out=ph[:], lhsT=w2[:], rhs=h1t[:, e * 512:(e + 1) * 512],
                             start=True, stop=True)
            nc.scalar.activation(out=h2t[:, e * 512:(e + 1) * 512], in_=ph[:],
                                 func=AF.Prelu, bias=b2c[:], alpha=alpha64[:])
        h3t = g["ht_pool"].tile([H, K * P], F32, tag="ht")
        for e in range(5):
            ph = pp_h.tile([H, 512], F32, tag="h")
            nc.tensor.matmul(out=ph[:], lhsT=w3[:], rhs=h2t[:, e * 512:(e + 1) * 512],
                             start=True, stop=True)
            nc.scalar.activation(out=h3t[:, e * 512:(e + 1) * 512], in_=ph[:],
                                 func=AF.Prelu, bias=b3c[:], alpha=alpha64[:])

        # aggregate: max over k (innermost stride-128 axis)
        ftile = g["f_pool"].tile([H, P], F32, tag="f")
        nc.vector.tensor_reduce(out=ftile[:],
                                in_=h3t[:].rearrange("h (k i) -> h i k", k=K),
                                axis=mybir.AxisListType.X, op=ALU.max)
        if out_d is None:
            nc.scalar.mul(out=x2t_next[0:H, t * P:(t + 1) * P], in_=ftile[:], mul=2.0)
        else:
            po = pp_tp.tile([P, H], F32, tag="tp")
            nc.tensor.transpose(out=po[:], in_=ftile[:], identity=ident[0:H, 0:H])
            nc.scalar.copy(out=g["vbuf_view"][:, t * H:(t + 1) * H], in_=po[:])
    if out_d is not None:
        nc.sync.dma_start(out_d[:].rearrange("(t p) f -> p t f", p=P),
                          g["vbuf_view"][:].rearrange("p (t f) -> p t f", f=H))


def build():
    nc = bacc.Bacc("TRN2", target_bir_lowering=False, debug=False,
                   num_swdge_queues=NSW)
    pos_d = nc.dram_tensor("pos", [N, 3], F32, kind="ExternalInput")
    wnames = {}
    for li, pfx in enumerate(("1", "2", "3")):
        d2 = 6 if li == 0 else 128
        wnames[f"w{pfx}1"] = nc.dram_tensor(f"w{pfx}1", [d2, H], F32, kind="ExternalInput")
        wnames[f"w{pfx}2"] = nc.dram_tensor(f"w{pfx}2", [H, H], F32, kind="ExternalInput")
        wnames[f"w{pfx}3"] = nc.dram_tensor(f"w{pfx}3", [H, H], F32, kind="ExternalInput")
        for j in ("1", "2", "3"):
            wnames[f"b{pfx}{j}"] = nc.dram_tensor(f"b{pfx}{j}", [H], F32, kind="ExternalInput")
    out_d = nc.dram_tensor("out", [N, H], F32, kind="ExternalOutput")
    v_ds = [nc.dram_tensor(f"vtab{li}", [N, H], F32) for li in range(3)]

    with tile.TileContext(nc) as tc:
        with tc.tile_pool(name="sb", bufs=1) as sb, \
             tc.tile_pool(name="s_pool", bufs=2) as s_pool, \
             tc.tile_pool(name="k_pool", bufs=2) as k_pool, \
             tc.tile_pool(name="h1_pool", bufs=2) as h1_pool, \
             tc.tile_pool(name="ht_pool", bufs=4) as ht_pool, \
             tc.tile_pool(name="f_pool", bufs=2) as f_pool, \
             tc.tile_pool(name="pp_s", bufs=3, space="PSUM") as pp_s, \
             tc.tile_pool(name="pp_tp", bufs=2, space="PSUM") as pp_tp, \
             tc.tile_pool(name="pp_h", bufs=2, space="PSUM") as pp_h:

            g = dict(sb=sb, s_pool=s_pool, k_pool=k_pool, h1_pool=h1_pool,
                     ht_pool=ht_pool, f_pool=f_pool,
                     pp_s=pp_s, pp_tp=pp_tp, pp_h=pp_h)

            ident = sb.tile([P, P], F32, tag="ident")
            make_identity(nc, ident)
            g["ident"] = ident
            ones1 = sb.tile([1, P], F32, tag="ones1")
            nc.vector.memset(ones1[:], 1.0)
            g["ones1"] = ones1
            alpha64 = sb.tile([H, 1], F32, tag="alpha64")
            nc.vector.memset(alpha64[:], SLOPE)
            g["alpha64"] = alpha64
            alpha128 = sb.tile([P, 1], F32, tag="alpha128")
            nc.vector.memset(alpha128[:], SLOPE)
            g["alpha128"] = alpha128
            base = sb.tile([P, NCH * 8], U32, tag="base")
            nc.gpsimd.iota(base[:], pattern=[[CS, NCH], [0, 8]], base=0,
                           channel_multiplier=0)
            g["base"] = base

            # layer inputs: 2*X^T ping-pong
            x2t_a = sb.tile([H, N], F32, tag="x2t_a")
            x2t_b = sb.tile([H, N], F32, tag="x2t_b")
            vbuf_view = sb.tile([P, NT * H], F32, tag="obuf")
            g["vbuf_view"] = vbuf_view

            # load pos -> 2*X^T (rows 0..2)
            xsb = sb.tile([P, NT * 3], F32, tag="xsb")
            nc.sync.dma_start(xsb[:].rearrange("p (t d) -> p t d", d=3),
                              pos_d[:].rearrange("(t p) d -> p t d", p=P))
            for t in range(NT):
                pt = pp_tp.tile([3, P], F32, tag="tp")
                nc.tensor.transpose(out=pt[:], in_=xsb[:, t * 3:(t + 1) * 3],
                                    identity=ident[:])
                nc.scalar.mul(out=x2t_a[0:3, t * P:(t + 1) * P], in_=pt[:], mul=2.0)

            _build_layer(nc, tc, g, 0, 3, x2t_a, x2t_b,
                         wnames["w11"], wnames["b11"], wnames["w12"], wnames["b12"],
                         wnames["w13"], wnames["b13"], v_ds[0])
            _build_layer(nc, tc, g, 1, H, x2t_b, x2t_a,
                         wnames["w21"], wnames["b21"], wnames["w22"], wnames["b22"],
                         wnames["w23"], wnames["b23"], v_ds[1])
            _build_layer(nc, tc, g, 2, H, x2t_a, None,
                         wnames["w31"], wnames["b31"], wnames["w32"], wnames["b32"],
                         wnames["w33"], wnames["b33"], v_ds[2], out_d=out_d)
    nc.finalize()
    return nc


def kernel(**inputs):
    if "nc" not in _CACHE:
        _CACHE["nc"] = build()
    nc = _CACHE["nc"]
    pos = np.ascontiguousarray(np.asarray(inputs["pos"], dtype=np.float32))
    weights = {k: np.ascontiguousarray(np.asarray(v, dtype=np.float32))
               for k, v in inputs.items() if k != "pos"}
    in_maps = []
    for b in range(B):
        m = {"pos": pos[b]}
        m.update(weights)
        in_maps.append(m)
    res = run_bass_kernel_spmd(nc, in_maps, core_ids=list(range(B)))
    out = np.stack([res.results[b]["out"] for b in range(B)], axis=0)
    return out


if __name__ == "__main__":
    rng = np.random.default_rng(0)
    fake = {"pos": rng.standard_normal((B, N, 3)).astype(np.float32)}
    for pfx in ("1", "2", "3"):
        d2 = 6 if pfx == "1" else 128
        fake[f"w{pfx}1"] = rng.standard_normal((d2, H)).astype(np.float32) * 0.2
        fake[f"w{pfx}2"] = rng.standard_normal((H, H)).astype(np.float32) * 0.12
        fake[f"w{pfx}3"] = rng.standard_normal((H, H)).astype(np.float32) * 0.12
        for j in ("1", "2", "3"):
            fake[f"b{pfx}{j}"] = np.zeros(H, np.float32)
    o = kernel(**fake)
    print("out", o.shape, o.dtype, float(np.abs(o).max()))
